# revision 35
# baseline (speedup 1.0000x reference)
"""AdversarialMorphingLoss — Trainium2 Bass kernel (8-core data parallel).

Full inputs arrive on the host; we shard the batch dim (B=4096) into 8
contiguous blocks of 512 rows, run one SPMD Bass program on all 8
NeuronCores, and each core returns the partial (un-normalized) sum of the
per-sample loss contribution over its 512 rows.  The host sums the 8
partials and divides by B.

Host-side prep (O(B) math + dtype casts):
  * the three [B, S] trace tensors are cast to fp16 (sizes/directions are
    exact; delays lose ~1e-4 relative — loss delta ~2e-7, tol 2e-2).
    Halves HBM traffic per core from 12.6 MB to 6.05 MB.
  * per-sample quantities depending only on [B] vectors are folded into a
    packed [B, 8] f32 tensor V:
      V0 = padding_norm*1500, V1 = delay_ms, V2 = mult*(100/S)/15,
      V4 = 0.2*(1-2*conf), V5 = 30*S/(100*mult)
    so ctot_b = relu(s0*V2 - 1) + V4*(s0 < V5) on device, while the
    per-sample base 0.5*sim + 0.3*eff + 0.2*conf^2 is summed on the host
    and added to the final scalar.

Device strategy (measured op rates on this toolchain):
  * DVE scalar_tensor_tensor w/ accum_out: 1x (2.29us per 2048-col tile)
    — the only fused compare+row-reduce DVE form that works on HW.
  * ScalarE ACTIVATE w/ accum_out: 1x @1.2GHz (2.0us) — Sign-based
    threshold counts.  16 fused count ops total, split 8/8:
      ACT: (sz>1400) and (dl<0.05) via Sign (sign-sum convention)
      DVE: sz[s]==sz[s-1] (is_equal), dir[s]!=dir[s-1] (not_equal)
  * all 16 input DMA triggers issue from the sync queue; tile 0's three
    tensors stream as 256KB halves so compute starts ~5us earlier.
  * every accumulator + last-col fixup lands in a [128, 64] f32 Rblock
    (16 slots x 4 tiles; slots 12-15 hold tile-0's second halves); the
    whole weighted merge is one tensor_tensor against a memset W tile +
    one strided tensor_reduce.
  * partition reduction via TensorE matmul into PSUM so the output DMA
    is one 4-byte descriptor (a [128,1] scatter costs ~4us completion).
"""

import numpy as np
from contextlib import ExitStack

import concourse.bass as bass
import concourse.bacc as bacc
import concourse.mybir as mybir
from concourse import tile
from concourse.bass_utils import run_bass_kernel_spmd

B, S = 4096, 2048
N_CORES = 8
BC = B // N_CORES          # 512 rows per core
P = 128                    # SBUF partitions
NT = BC // P               # 4 tiles of 128 rows per core
NR = 16                    # Rblock slots per tile
H = S // 2

F32 = mybir.dt.float32
F16 = mybir.dt.float16
U8 = mybir.dt.uint8
ALU = mybir.AluOpType
ACTF = mybir.ActivationFunctionType

# per-profile targets (match reference.py)
TARGET_DELAY = np.array([2.0, 1.0, 0.5, 5.0, 3.0], dtype=np.float32)
TARGET_PAD = np.array([0.08, 0.12, 0.05, 0.15, 0.10], dtype=np.float32)
CONFIG_MULT = np.array([1.0, 1.3, 1.6, 2.0], dtype=np.float32)

_NC_CACHE = None
LAST_RESULTS = None        # BassKernelResults of the last kernel() call


def _patch_drain(tc, out_dma_holder):
    """Slim TileContext's exit drain (controlled by KERNEL_DRAIN_MODE):
    'full'     stock ending (drain + EVSEM barrier + sem clear + barrier)
    'nobar2'   stock minus the trailing all-engine barrier
    'plainsem' plain-semaphore ending (see baseline notes).
    """
    import os
    import re
    import types
    from concourse.vector_clock import ScopedClock

    mode = os.environ.get("KERNEL_DRAIN_MODE", "nobar2")
    if mode == "full":
        return

    def _slim(self, tick_clock, wait_clock):
        nc = self.nc
        if mode == "plainsem":
            totals = {}
            upd_re = re.compile(r"update:S\[([A-Za-z0-9_]+)\](?:\+\+|\+=)(\d+)")
            for bb in nc.main_func.blocks:
                for ins in bb.instructions:
                    for mm in upd_re.finditer(str(ins)):
                        totals[mm.group(1)] = totals.get(mm.group(1), 0) + int(mm.group(2))
            by_name = {h.name: h for h in self.sems.allocated().values()}
            waits = [(h, totals[name]) for name, h in sorted(by_name.items())
                     if totals.get(name, 0) > 0]
            for eng in nc.engines.values():
                for h, total in waits:
                    eng.wait_ge(h, total)
            popped = nc._tile_sem_poison_stack.pop()
            assert popped is self._sem_poison
            nc.clear_and_free_semaphores(
                list(self.sems.allocated().values()))
            return
        drain_inst = nc.sync.drain()
        wait_clock.add_sem_waits(
            drain_inst.ins, ScopedClock({None: tick_clock.global_clock}))
        nc.all_engine_barrier()
        popped = nc._tile_sem_poison_stack.pop()
        assert popped is self._sem_poison
        nc.clear_and_free_semaphores(list(self.sems.allocated().values()))

    tc._drain_and_barrier = types.MethodType(_slim, tc)


def _build_nc() -> bass.Bass:
    nc = bacc.Bacc()

    sz_h = nc.declare_dram_parameter("raw_sizes", [BC, S], F16, isOutput=False)
    dl_h = nc.declare_dram_parameter("raw_delays", [BC, S], F16, isOutput=False)
    dr_h = nc.declare_dram_parameter("raw_directions", [BC, S], U8, isOutput=False)
    v_h = nc.declare_dram_parameter("vpack", [BC, 8], F32, isOutput=False)
    out_h = nc.declare_dram_parameter("partial", [1, NT], F32, isOutput=True)

    out_dma_holder = []
    with tile.TileContext(nc) as tc, ExitStack() as ctx:
        _patch_drain(tc, out_dma_holder)
        sm = ctx.enter_context(tc.tile_pool(name="sm", bufs=1))
        scr = ctx.enter_context(tc.tile_pool(name="scr", bufs=2))
        pp = ctx.enter_context(tc.tile_pool(name="pp", bufs=1, space="PSUM"))

        # big input tensors: one SBUF tensor per input, tile t = cols [t*S,(t+1)*S)
        SZ = sm.tile([P, NT * S], F16, tag="SZ", name="SZ")
        DL = sm.tile([P, NT * S], F16, tag="DL", name="DL")
        DR = sm.tile([P, NT * S], U8, tag="DR", name="DR")
        V = sm.tile([P, NT * 8], F32, tag="V", name="V")
        Rb = sm.tile([P, NT * NR], F32, tag="Rb", name="Rb")
        W = sm.tile([P, NT * NR], F32, tag="W", name="W")

        _consts = {}

        def constv(val):
            if val not in _consts:
                cname = f"cst{len(_consts)}"
                ct = sm.tile([P, 1], F32, tag=cname, name=cname)
                nc.vector.memset(ct[:, :], val)
                _consts[val] = ct[:, :]
            return _consts[val]

        # DRAM views: tile t holds rows r = p*NT + t
        sz_t = sz_h[:, :].rearrange("(p t) s -> t p s", t=NT)
        dl_t = dl_h[:, :].rearrange("(p t) s -> t p s", t=NT)
        dr_t = dr_h[:, :].rearrange("(p t) s -> t p s", t=NT)
        v_d = v_h[:, :].rearrange("(p t) v -> p (t v)", t=NT)

        def szs(t):
            return slice(t * S, (t + 1) * S)

        # ---- DMA triggers: all on the sync HWDGE ring, arrival order =
        # compute order; tile 0 streams as halves for a fast pipeline fill.
        Q = H // 2
        nc.gpsimd.dma_start(V[:, :], v_d)
        # tile-0 sizes stream as 4 quarters split across BOTH HWDGE rings so
        # descriptor generation and completion receipts overlap — the first
        # pair lands ~1us earlier, unblocking both engines' first ops
        nc.sync.dma_start(SZ[:, 0:Q], sz_t[0][:, 0:Q])
        nc.scalar.dma_start(SZ[:, Q:H], sz_t[0][:, Q:H])
        nc.sync.dma_start(SZ[:, H:H + Q], sz_t[0][:, H:H + Q])
        nc.scalar.dma_start(SZ[:, H + Q:S], sz_t[0][:, H + Q:S])
        # all sizes first (they feed both engines' phase-1), then dirs/delays;
        # dirs tiles 1-3 coalesced (rows are DRAM-contiguous across t)
        dr_flat = dr_h[:, :].rearrange("(p t) s -> p (t s)", t=NT)
        nc.sync.dma_start(SZ[:, szs(1)], sz_t[1])
        nc.sync.dma_start(SZ[:, szs(2)], sz_t[2])
        nc.sync.dma_start(SZ[:, szs(3)], sz_t[3])
        nc.sync.dma_start(DR[:, szs(0)], dr_t[0])
        nc.sync.dma_start(DL[:, szs(0)], dl_t[0])
        nc.sync.dma_start(DR[:, S:4 * S], dr_flat[:, S:4 * S])
        nc.sync.dma_start(DL[:, szs(1)], dl_t[1])
        nc.sync.dma_start(DL[:, szs(2)], dl_t[2])
        nc.sync.dma_start(DL[:, szs(3)], dl_t[3])

        # ---- W weight tile + Rblock const columns (gpsimd memsets) ----
        # Rblock slot r semantics (per tile-column t):
        #  0: A = sum sign(sz-1400.5)        w=0.3   (0.6 * 1/2)
        #  1: B = sum sign(0.05-dl)          w=0.2   (0.4 * 1/2)
        #  2: C = sum is_equal(sz_s,sz_s-1)  w=0.2
        #  3: D = sum not_equal(dr_s,dr_s-1) w=0.1
        #  4: g1r=szlast>1400  w=-0.6    5: g1m=szmod>1400  w=+0.6
        #  6: l2r=dllast<0.05  w=-0.4    7: l2m=dlmod<0.05  w=+0.4
        #  8: e3r=szlast==szprev w=-0.2  9: e3m=|szmod-szprev|<0.5 w=+0.2
        # 10: const 1.0, w = 0.6*1024 + 0.4*1024 + 0.1   11: unused
        # 12-15: tile-0 second halves of slots 0-3 (same weights)
        Wr = W[:, :].rearrange("p (t r) -> r p t", r=NR)
        Rr = Rb[:, :].rearrange("p (t r) -> r p t", r=NR)
        g = nc.gpsimd
        g.memset(Wr[0], 0.3)
        g.memset(Wr[1], 0.2)
        g.memset(Wr[2], 0.2)
        g.memset(Wr[3], 0.1)
        for r in range(4, 10):
            g.memset(Wr[r], 0.0)
            g.memset(Rr[r], 0.0)
        g.memset(Wr[10], 0.6 * 1024.0 + 0.4 * 1024.0 + 0.1)
        g.memset(Rr[10], 1.0)
        g.memset(Wr[12][:, 0:1], 0.3)
        g.memset(Wr[13][:, 0:1], 0.2)
        g.memset(Wr[14][:, 0:1], 0.2)
        g.memset(Wr[11], 0.0)
        g.memset(Wr[12][:, 1:4], 0.0)
        g.memset(Wr[13][:, 1:4], 0.0)
        g.memset(Wr[14][:, 1:4], 0.0)
        g.memset(Wr[15], 0.0)
        for r in range(11, 16):
            g.memset(Rr[r], 0.0)

        v = nc.vector

        def rslot(t, r):
            c = t * NR + r
            return Rb[:, c:c + 1]

        # strided per-sample views from the packed V tensor
        Vr = V[:, :].rearrange("p (t v) -> v p t", v=8)
        mlt2v, fv, w1v, thr30v = Vr[2], Vr[3], Vr[4], Vr[5]

        # ---- fused count ops (accum_out -> Rblock) ----
        def act_sign_sz(cs, slot):
            o = scr.tile([P, cs.stop - cs.start], F16, tag="osg")
            nc.scalar.activation(o[:, :], SZ[:, cs], ACTF.Sign,
                                 bias=constv(-1400.5), scale=1.0,
                                 accum_out=slot)

        def act_sign_dl(cs, slot):
            o = scr.tile([P, cs.stop - cs.start], F16, tag="osg")
            nc.scalar.activation(o[:, :], DL[:, cs], ACTF.Sign,
                                 bias=constv(0.05), scale=-1.0,
                                 accum_out=slot)

        def dve_eq_sz(lo, hi, slot):
            o = scr.tile([P, hi - lo], F16, tag="oeq")
            v.scalar_tensor_tensor(
                o[:, :], SZ[:, lo + 1:hi + 1], 0.0, SZ[:, lo:hi],
                ALU.bypass, ALU.is_equal, accum_out=slot)

        def dve_ne_dr(lo, hi, slot):
            o = scr.tile([P, hi - lo], U8, tag="one")
            v.scalar_tensor_tensor(
                o[:, :], DR[:, lo + 1:hi + 1], 0.0, DR[:, lo:hi],
                ALU.bypass, ALU.not_equal, accum_out=slot)

        # scalar engine queue, in data-arrival order.  The dummy Sign on a
        # const forces the ACT table load at kernel start (no data deps)
        # instead of gating the first real ACTIVATE.
        dmy = sm.tile([P, 1], F16, tag="dmy", name="dmy")
        nc.scalar.activation(dmy[:, :], constv(0.0), ACTF.Sign,
                             bias=constv(-1400.5), scale=1.0)
        act_sign_sz(slice(0, H), rslot(0, 0))
        act_sign_sz(slice(H, S), rslot(0, 12))
        act_sign_sz(slice(S, 2 * S), rslot(1, 0))
        act_sign_sz(slice(2 * S, 3 * S), rslot(2, 0))
        act_sign_sz(slice(3 * S, 4 * S), rslot(3, 0))
        act_sign_dl(slice(0, S), rslot(0, 1))
        act_sign_dl(slice(S, 2 * S), rslot(1, 1))
        act_sign_dl(slice(2 * S, 3 * S), rslot(2, 1))
        act_sign_dl(slice(3 * S, 4 * S), rslot(3, 1))

        # vector engine queue, in data-arrival order
        dve_eq_sz(0, Q - 1, rslot(0, 2))            # pairs s=1..Q-1
        dve_eq_sz(Q - 1, H - 1, rslot(0, 13))       # pairs s=Q..H-1
        dve_eq_sz(H - 1, S - 1, rslot(0, 14))       # pairs s=H..S-1
        dve_eq_sz(S, 2 * S - 1, rslot(1, 2))
        dve_eq_sz(2 * S, 3 * S - 1, rslot(2, 2))
        dve_eq_sz(3 * S, 4 * S - 1, rslot(3, 2))
        dve_ne_dr(0, S - 1, rslot(0, 3))
        dve_ne_dr(S, 2 * S - 1, rslot(1, 3))
        dve_ne_dr(2 * S, 3 * S - 1, rslot(2, 3))

        dve_ne_dr(3 * S, 4 * S - 1, rslot(3, 3))


        # ---- merge: scores per sample, then loss terms.
        # V2 = mult*(100/S)/15 so dpi-term = relu(s0*V2 - 1) = (2/30)*relu(scores-15);
        # V5 = 30*S/(100*mult) so ev = (s0 < V5) = (scores < 30).  The per-sample
        # base 0.5*sim+0.3*eff+0.2*conf^2 is summed on the host.
        M = sm.tile([P, NT * NR], F32, tag="M", name="M")
        v.tensor_tensor(M[:, :], Rb[:, :], W[:, :], ALU.mult)
        s0 = sm.tile([P, NT], F32, tag="s0", name="s0")
        v.tensor_reduce(s0[:, :], M[:, :].rearrange("p (t r) -> p t r", r=NR),
                        axis=mybir.AxisListType.X, op=ALU.add)
        v.tensor_tensor(s0[:, :], s0[:, :], fv, ALU.add)
        dpi = sm.tile([P, NT], F32, tag="dpi", name="dpi")
        v.tensor_tensor(dpi[:, :], s0[:, :], mlt2v, ALU.mult)
        v.tensor_scalar(dpi[:, :], dpi[:, :], 1.0, -1.0, ALU.max, ALU.add)
        ev = sm.tile([P, NT], F32, tag="ev", name="ev")
        v.tensor_tensor(ev[:, :], s0[:, :], thr30v, ALU.is_lt)
        u1 = sm.tile([P, NT], F32, tag="u1", name="u1")
        v.tensor_tensor(u1[:, :], ev[:, :], w1v, ALU.mult)
        ctot = sm.tile([P, NT], F32, tag="ctot", name="ctot")
        v.tensor_tensor(ctot[:, :], dpi[:, :], u1[:, :], ALU.add)

        # partition reduction on the (idle) tensor engine -> [1,NT] PSUM;
        # the host sums the NT per-tile partials (skips a DVE reduce)
        ps = pp.tile([1, NT], F32, tag="ps", name="ps")
        nc.tensor.matmul(ps[:, :], constv(1.0), ctot[:, :], start=True, stop=True)
        outsb = sm.tile([1, NT], F32, tag="outsb", name="outsb")
        v.tensor_copy(outsb[:, :], ps[:, :])
        out_dma_holder.append(nc.sync.dma_start(out_h[:, :], outsb[:, :]))

    nc.finalize()
    return nc


def _get_nc() -> bass.Bass:
    global _NC_CACHE
    if _NC_CACHE is None:
        _NC_CACHE = _build_nc()
    return _NC_CACHE


def kernel(raw_sizes, raw_delays, raw_directions, delay_ms, padding_norm,
           confidence, profile_ids, trace=False, tmpdir=None):
    global LAST_RESULTS
    sz16 = np.asarray(raw_sizes).astype(np.float16)
    dl16 = np.asarray(raw_delays, dtype=np.float32).astype(np.float16)
    dr8 = np.asarray(raw_directions).astype(np.uint8)
    dms = np.asarray(delay_ms, dtype=np.float32)
    pad = np.asarray(padding_norm, dtype=np.float32)
    conf = np.asarray(confidence, dtype=np.float32)
    pid = np.asarray(profile_ids).astype(np.int64)

    td = TARGET_DELAY[pid]
    tp = TARGET_PAD[pid]
    mult = CONFIG_MULT[pid % 4]
    sim = np.abs(dms - td) + np.abs(pad - tp)
    eff = np.maximum(dms - 20.0, 0.0) / 20.0 + np.maximum(pad - 0.3, 0.0)
    # last-packet morphing fixups are O(B): computed here from the same
    # fp16-cast values the device streams, exactly as the reference does
    szl = sz16[:, -1].astype(np.float32)
    szp = sz16[:, -2].astype(np.float32)
    dll = dl16[:, -1].astype(np.float32)
    padx = (pad * 1500.0).astype(np.float32)
    szmod = np.minimum(szl + padx, 1500.0)
    dlmod = dll + dms
    ffix = (0.6 * ((szmod > 1400.0).astype(np.float32) - (szl > 1400.0))
            + 0.4 * ((dlmod < 0.05).astype(np.float32) - (dll < 0.05))
            + 0.2 * ((np.abs(szmod - szp) < 0.5).astype(np.float32) - (szl == szp)))
    vpack = np.zeros((B, 8), dtype=np.float32)
    vpack[:, 2] = mult * (100.0 / S) / 15.0
    vpack[:, 3] = ffix
    vpack[:, 4] = 0.2 * (1.0 - 2.0 * conf)
    vpack[:, 5] = 30.0 * S / (100.0 * mult)
    e0_sum = float(np.sum(0.5 * sim + 0.3 * eff + 0.2 * conf * conf,
                          dtype=np.float64))

    nc = _get_nc()
    in_maps = []
    for i in range(N_CORES):
        r = slice(i * BC, (i + 1) * BC)
        in_maps.append({
            "raw_sizes": sz16[r],
            "raw_delays": dl16[r],
            "raw_directions": dr8[r],
            "vpack": vpack[r],
        })

    LAST_RESULTS = run_bass_kernel_spmd(nc, in_maps, list(range(N_CORES)),
                                        trace=trace, tmpdir=tmpdir)
    partials = [LAST_RESULTS.results[i]["partial"] for i in range(N_CORES)]
    total = float(np.sum(np.stack(partials), dtype=np.float64)) + e0_sum
    return np.float32(total / B)
